# revision 12
# baseline (speedup 1.0000x reference)
"""Causal self-attention (B=2, T=2048, D=2048, H=16) on 8 Trainium2 cores.

Sharding: 2-way batch x 4-way head-group tensor parallel. Core c owns
batch b = c//4 and heads [4*(c%4), 4*(c%4)+4). Each core computes its
heads' Q/K/V projections, causal attention, and a row-sharded partial of
the output projection; the host sums the 4 partials per batch (all-reduce
on host) and concatenates k/v along heads.

All matmuls run in bf16 with fp32 PSUM accumulation; softmax runs in
fp32 on the scalar engine (no max subtraction needed: |scores| <= ~7).
"""

import math

import ml_dtypes
import numpy as np

BF16 = ml_dtypes.bfloat16

B, T, D, H = 2, 2048, 2048, 16
HD = D // H            # 128 head dim
NCORES = 8
NHG = 4                # head groups (cores per batch)
HPC = H // NHG         # 4 heads per core
DL = HPC * HD          # 512 projection cols per core
P = 128
TCH = 512              # t-chunk (matmul moving width)
NTCH = T // TCH        # 4
CCH = D // P           # 16 contraction chunks

_CACHE = {}


def _build_nc():
    from contextlib import ExitStack

    import concourse.tile as tile
    import concourse.mybir as mybir
    from concourse import bacc

    dt = mybir.dt
    f32 = dt.float32
    b16 = dt.bfloat16

    nc = bacc.Bacc(
        "TRN2", target_bir_lowering=False, debug=False,
        enable_asserts=False, num_devices=NCORES,
    )

    xT = nc.dram_tensor("xT", [D, T], b16, kind="ExternalInput").ap()
    wq = nc.dram_tensor("wq", [P, CCH, DL], b16, kind="ExternalInput").ap()
    wkv = nc.dram_tensor("wkv", [P, CCH, 2 * DL], b16, kind="ExternalInput").ap()
    wo = nc.dram_tensor("wo", [P, HPC, D], b16, kind="ExternalInput").ap()
    bqv = nc.dram_tensor("bqv", [P, HPC], f32, kind="ExternalInput").ap()
    ident = nc.dram_tensor("ident", [P, P], b16, kind="ExternalInput").ap()
    mask = nc.dram_tensor("mask", [P, P], b16, kind="ExternalInput").ap()

    out_p = nc.dram_tensor("out_p", [T, D], f32, kind="ExternalOutput").ap()
    k_out = nc.dram_tensor("k_out", [HPC, T, HD], f32, kind="ExternalOutput").ap()
    v_out = nc.dram_tensor("v_out", [HPC, T, HD], f32, kind="ExternalOutput").ap()

    Exp = mybir.ActivationFunctionType.Exp
    mult = mybir.AluOpType.mult
    add = mybir.AluOpType.add

    with tile.TileContext(nc) as tc, ExitStack() as ctx:
        consts = ctx.enter_context(tc.tile_pool(name="consts", bufs=1))
        persist = ctx.enter_context(tc.tile_pool(name="persist", bufs=1))
        xtp = ctx.enter_context(tc.tile_pool(name="xtp", bufs=26))
        kvf = ctx.enter_context(tc.tile_pool(name="kvf", bufs=6))
        k16p = ctx.enter_context(tc.tile_pool(name="k16p", bufs=3))
        expp = ctx.enter_context(tc.tile_pool(name="expp", bufs=18))
        anp = ctx.enter_context(tc.tile_pool(name="anp", bufs=3))
        recp = ctx.enter_context(tc.tile_pool(name="recp", bufs=4))
        outp = ctx.enter_context(tc.tile_pool(name="outp", bufs=4))
        ps_mm = ctx.enter_context(tc.tile_pool(name="ps_mm", bufs=3, space="PSUM"))
        ps_sc = ctx.enter_context(tc.tile_pool(name="ps_sc", bufs=2, space="PSUM"))
        ps_sm = ctx.enter_context(tc.tile_pool(name="ps_sm", bufs=3, space="PSUM"))

        mask_sb = consts.tile([P, P], b16)
        nc.sync.dma_start(mask_sb[:], mask)
        # Only what the first Q^T matmuls need is loaded up front; wkv/wo
        # are deferred below so they don't steal HBM bandwidth at the head.
        wq_sb = consts.tile([P, CCH, DL], b16)
        for dd in range(HPC):
            nc.scalar.dma_start(wq_sb[:, :, dd * P:(dd + 1) * P],
                                wq[:, :, dd * P:(dd + 1) * P])
        bq_sb = consts.tile([P, HPC], f32)
        nc.scalar.dma_start(bq_sb[:], bqv)
        id_sb = consts.tile([P, P], b16)
        nc.sync.dma_start(id_sb[:], ident)
        wkv_sb = consts.tile([P, CCH, 2 * DL], b16)
        wo_sb = consts.tile([P, HPC, D], b16)

        # Persistent per-(head) state, bf16: Q^T, K^T as [d, t]; V (+ones col)
        # as [t-blocks, d+1]; attn^T as [d, t].
        qT = persist.tile([P, HPC, T], b16)
        kT = persist.tile([P, HPC, T], b16)
        vA = persist.tile([P, HPC, T // P, HD + 4], b16)
        aT = persist.tile([P, HPC, T], b16)
        nc.vector.memset(vA[:, :, :, HD:HD + 1], 1.0)

        # ~4us of dummy matmuls to flip the PE HAM clock-gate to 8/8 while
        # the weight/x DMAs stream in.
        for w in range(40):
            psw = ps_sm.tile([P, HD + 4], f32, tag="sm")
            nc.tensor.matmul(psw[:, :P], mask_sb[:], mask_sb[:],
                             start=True, stop=True)

        for tj in range(NTCH):
            ts = slice(tj * TCH, (tj + 1) * TCH)
            # ---- x^T tiles for this t-chunk
            xts = []
            for cc in range(CCH):
                xt = xtp.tile([P, TCH], b16, tag="xt")
                nc.sync.dma_start(
                    xt[:], xT[cc * P:(cc + 1) * P, ts])
                xts.append(xt)

            # ---- Q^T projection: [d-block, t-chunk] += Wq_cc^T @ xT_cc
            for dd in range(HPC):
                psq = ps_mm.tile([P, TCH], f32, tag="mm")
                for cc in range(CCH):
                    nc.tensor.matmul(
                        psq[:], wq_sb[:, cc, dd * P:(dd + 1) * P], xts[cc][:],
                        start=(cc == 0), stop=(cc == CCH - 1))
                nc.vector.tensor_scalar(
                    qT[:, dd, ts], psq[:], bq_sb[:, dd:dd + 1], None, add)

            if tj == 0:
                nc.scalar.dma_start(wkv_sb[:], wkv)
                nc.scalar.dma_start(wo_sb[:], wo)

            # ---- K, V natural projection per t-block
            def flush_ktr(pending):
                for kk, ptb in pending:
                    nc.scalar.dma_start_transpose(
                        kT[:, :, ptb * P:(ptb + 1) * P], kk[:])
                pending.clear()

            pending_ktr = []
            for tl in range(TCH // P):
                tb = tj * (TCH // P) + tl
                bs = slice(tl * P, (tl + 1) * P)
                psk = ps_mm.tile([P, TCH], f32, tag="mm")
                psv = ps_mm.tile([P, TCH], f32, tag="mm")
                for cc in range(CCH):
                    nc.tensor.matmul(
                        psk[:], xts[cc][:, bs], wkv_sb[:, cc, :DL],
                        start=(cc == 0), stop=(cc == CCH - 1))
                for cc in range(CCH):
                    nc.tensor.matmul(
                        psv[:], xts[cc][:, bs], wkv_sb[:, cc, DL:],
                        start=(cc == 0), stop=(cc == CCH - 1))
                # k transposes of the PREVIOUS t-block: their k16 copy is
                # long done, so these matmuls issue without waiting on DVE.
                flush_ktr(pending_ktr)

                kf = kvf.tile([P, DL], f32, tag="kvf")
                nc.vector.tensor_copy(out=kf[:], in_=psk[:])
                nc.sync.dma_start(
                    k_out[:, tb * P:(tb + 1) * P, :].rearrange("h t d -> t h d"),
                    kf[:].rearrange("t (h d) -> t h d", d=HD))
                vf = kvf.tile([P, DL], f32, tag="kvf")
                nc.vector.tensor_copy(out=vf[:], in_=psv[:])
                nc.sync.dma_start(
                    v_out[:, tb * P:(tb + 1) * P, :].rearrange("h t d -> t h d"),
                    vf[:].rearrange("t (h d) -> t h d", d=HD))

                k16 = k16p.tile([P, DL], b16, tag="k16")
                nc.vector.tensor_copy(out=k16[:], in_=psk[:])
                nc.vector.tensor_copy(
                    out=vA[:, :, tb, :HD],
                    in_=psv[:].rearrange("t (h d) -> t h d", d=HD))
                pending_ktr.append((k16, tb))
            flush_ktr(pending_ktr)

            # ---- attention for q-chunk jq = tj (needs kv t-blocks <= this chunk)
            nkb = 4 * tj + 4

            def flush_atr(pending):
                for pan4, ph in pending:
                    nc.scalar.dma_start_transpose(
                        aT[:, ph, ts].rearrange("d (g q) -> d g q", q=P),
                        pan4[:])
                pending.clear()

            pending_atr = []
            for h in range(HPC):
                ets = []
                for i in range(nkb):
                    pss = ps_sc.tile([P, TCH], f32, tag="sc")
                    nc.tensor.matmul(
                        pss[:], kT[:, h, i * P:(i + 1) * P], qT[:, h, ts],
                        start=True, stop=True)
                    et = expp.tile([P, TCH], b16, tag="exp")
                    nc.scalar.activation(et[:], pss[:], Exp)
                    if i >= 4 * tj:
                        dl = i - 4 * tj
                        nc.vector.tensor_tensor(
                            out=et[:, dl * P:(dl + 1) * P],
                            in0=et[:, dl * P:(dl + 1) * P],
                            in1=mask_sb[:], op=mult)
                    ets.append(et)
                # previous head's attn transposes: their normalize chain is
                # done by now, so these don't stall the PE stream.
                flush_atr(pending_atr)
                an4 = anp.tile([P, TCH], b16, tag="an")
                for gl in range(4):
                    g = 4 * tj + gl
                    psp = ps_sm.tile([P, HD + 4], f32, tag="sm")
                    for i in range(g + 1):
                        nc.tensor.matmul(
                            psp[:, :HD + 1],
                            ets[i][:, gl * P:(gl + 1) * P],
                            vA[:, h, i, :HD + 1],
                            start=(i == 0), stop=(i == g))
                    rec = recp.tile([P, 1], f32, tag="rec")
                    nc.vector.reciprocal(rec[:], psp[:, HD:HD + 1])
                    nc.vector.tensor_scalar(
                        an4[:, gl * P:(gl + 1) * P], psp[:, :HD], rec[:],
                        None, mult)
                pending_atr.append((an4, h))
            flush_atr(pending_atr)

            # ---- output projection, one chunk behind (fills attention
            # bubbles of the current chunk)
            wo_chunks = [tj - 1] if tj < NTCH - 1 else [tj - 1, tj]
            for wj in wo_chunks:
                if wj < 0:
                    continue
                for tl in range(TCH // P):
                    tb = wj * (TCH // P) + tl
                    for nn in range(D // TCH):
                        pso = ps_mm.tile([P, TCH], f32, tag="mm")
                        for h in range(HPC):
                            nc.tensor.matmul(
                                pso[:], aT[:, h, tb * P:(tb + 1) * P],
                                wo_sb[:, h, nn * TCH:(nn + 1) * TCH],
                                start=(h == 0), stop=(h == HPC - 1))
                        ob = outp.tile([P, TCH], f32, tag="ob")
                        nc.vector.tensor_copy(out=ob[:], in_=pso[:])
                        nc.sync.dma_start(
                            out_p[tb * P:(tb + 1) * P,
                                  nn * TCH:(nn + 1) * TCH],
                            ob[:])

    nc.compile()
    return nc


def _prep_inputs(x, Wq, bq, Wk, bk, Wv, bv, Wo, bo):
    scale = 1.0 / math.sqrt(HD)
    x = np.asarray(x, np.float32)
    xT_b = [np.ascontiguousarray(x[b].T).astype(BF16) for b in range(B)]

    ident = np.eye(P, dtype=np.float32).astype(BF16)
    # mask[k, q] = 1 where q >= k (upper triangular incl diagonal)
    mask = np.triu(np.ones((P, P), np.float32)).astype(BF16)

    per_group = []
    for hg in range(NHG):
        cols = slice(hg * DL, (hg + 1) * DL)
        wq_c = np.ascontiguousarray(
            (np.asarray(Wq[:, cols], np.float32) * scale)
            .reshape(CCH, P, DL).transpose(1, 0, 2)).astype(BF16)
        wkv_c = np.zeros((P, CCH, 2 * DL), np.float32)
        wkv_c[:, :, :DL] = np.asarray(Wk[:, cols], np.float32).reshape(
            CCH, P, DL).transpose(1, 0, 2)
        wkv_c[:, :, DL:] = np.asarray(Wv[:, cols], np.float32).reshape(
            CCH, P, DL).transpose(1, 0, 2)
        wkv_c = np.ascontiguousarray(wkv_c).astype(BF16)
        wo_c = np.ascontiguousarray(
            np.asarray(Wo[cols, :], np.float32)
            .reshape(HPC, HD, D).transpose(1, 0, 2)).astype(BF16)
        bq_c = np.ascontiguousarray(
            (np.asarray(bq[cols], np.float32) * scale).reshape(HPC, HD).T)
        per_group.append((wq_c, wkv_c, wo_c, bq_c))

    in_maps = []
    for c in range(NCORES):
        b, hg = c // NHG, c % NHG
        wq_c, wkv_c, wo_c, bq_c = per_group[hg]
        in_maps.append({
            "xT": xT_b[b], "wq": wq_c, "wkv": wkv_c, "wo": wo_c,
            "bqv": bq_c, "ident": ident, "mask": mask,
        })
    return in_maps


def _run(inputs, trace=False):
    import concourse.bass_utils as bass_utils

    if "nc" not in _CACHE:
        _CACHE["nc"] = _build_nc()
    nc = _CACHE["nc"]
    in_maps = _prep_inputs(**inputs)
    res = bass_utils.run_bass_kernel_spmd(
        nc, in_maps, core_ids=list(range(NCORES)), trace=trace)

    bo = np.asarray(inputs["bo"], np.float32)
    bk = np.asarray(inputs["bk"], np.float32)
    bv = np.asarray(inputs["bv"], np.float32)
    Wo = np.asarray(inputs["Wo"], np.float32)
    # K/V biases are skipped on device: softmax is shift-invariant in the
    # K bias, and sum(p)=1 makes the V bias contribute exactly bv @ Wo.
    out_bias = bo + bv @ Wo
    bk_h = bk.reshape(H, 1, HD)
    bv_h = bv.reshape(H, 1, HD)
    out = np.empty((B, T, D), np.float32)
    k = np.empty((B, H, T, HD), np.float32)
    v = np.empty((B, H, T, HD), np.float32)
    for b in range(B):
        acc = None
        for hg in range(NHG):
            r = res.results[b * NHG + hg]
            acc = r["out_p"] if acc is None else acc + r["out_p"]
            hs = slice(hg * HPC, (hg + 1) * HPC)
            k[b, hs] = r["k_out"] + bk_h[hs]
            v[b, hs] = r["v_out"] + bv_h[hs]
        out[b] = acc + out_bias
    return (out, k, v), res


def kernel(**inputs):
    outs, _ = _run(inputs, trace=False)
    return outs


# revision 13
# speedup vs baseline: 1.1334x; 1.1334x over previous
"""Causal self-attention (B=2, T=2048, D=2048, H=16) on 8 Trainium2 cores.

Sharding: 2-way batch x 4-way head-group tensor parallel. Core c owns
batch b = c//4 and heads [4*(c%4), 4*(c%4)+4). Each core computes its
heads' Q/K/V projections, causal attention, and a row-sharded partial of
the output projection; the host sums the 4 partials per batch (all-reduce
on host) and concatenates k/v along heads.

All matmuls run in bf16 with fp32 PSUM accumulation; softmax runs in
fp32 on the scalar engine (no max subtraction needed: |scores| <= ~7).
"""

import math

import ml_dtypes
import numpy as np

BF16 = ml_dtypes.bfloat16

B, T, D, H = 2, 2048, 2048, 16
HD = D // H            # 128 head dim
NCORES = 8
NHG = 4                # head groups (cores per batch)
HPC = H // NHG         # 4 heads per core
DL = HPC * HD          # 512 projection cols per core
P = 128
TCH = 512              # t-chunk (matmul moving width)
NTCH = T // TCH        # 4
CCH = D // P           # 16 contraction chunks

_CACHE = {}


def _build_nc():
    from contextlib import ExitStack

    import concourse.tile as tile
    import concourse.mybir as mybir
    from concourse import bacc

    dt = mybir.dt
    f32 = dt.float32
    b16 = dt.bfloat16

    nc = bacc.Bacc(
        "TRN2", target_bir_lowering=False, debug=False,
        enable_asserts=False, num_devices=NCORES,
    )

    xT = nc.dram_tensor("xT", [D, T], b16, kind="ExternalInput").ap()
    wq = nc.dram_tensor("wq", [P, CCH, DL], b16, kind="ExternalInput").ap()
    wkv = nc.dram_tensor("wkv", [P, CCH, 2 * DL], b16, kind="ExternalInput").ap()
    wo = nc.dram_tensor("wo", [P, HPC, D], b16, kind="ExternalInput").ap()
    bqv = nc.dram_tensor("bqv", [P, HPC], f32, kind="ExternalInput").ap()
    ident = nc.dram_tensor("ident", [P, P], b16, kind="ExternalInput").ap()
    mask = nc.dram_tensor("mask", [P, P], b16, kind="ExternalInput").ap()

    out_p = nc.dram_tensor("out_p", [T, D], f32, kind="ExternalOutput").ap()
    k_out = nc.dram_tensor("k_out", [HPC, T, HD], f32, kind="ExternalOutput").ap()
    v_out = nc.dram_tensor("v_out", [HPC, T, HD], f32, kind="ExternalOutput").ap()

    Exp = mybir.ActivationFunctionType.Exp
    mult = mybir.AluOpType.mult
    add = mybir.AluOpType.add

    with tile.TileContext(nc) as tc, ExitStack() as ctx:
        consts = ctx.enter_context(tc.tile_pool(name="consts", bufs=1))
        persist = ctx.enter_context(tc.tile_pool(name="persist", bufs=1))
        xtp = ctx.enter_context(tc.tile_pool(name="xtp", bufs=26))
        kvf = ctx.enter_context(tc.tile_pool(name="kvf", bufs=6))
        k16p = ctx.enter_context(tc.tile_pool(name="k16p", bufs=3))
        expp = ctx.enter_context(tc.tile_pool(name="expp", bufs=18))
        anp = ctx.enter_context(tc.tile_pool(name="anp", bufs=10))
        recp = ctx.enter_context(tc.tile_pool(name="recp", bufs=4))
        outp = ctx.enter_context(tc.tile_pool(name="outp", bufs=4))
        ps_mm = ctx.enter_context(tc.tile_pool(name="ps_mm", bufs=3, space="PSUM"))
        ps_sc = ctx.enter_context(tc.tile_pool(name="ps_sc", bufs=2, space="PSUM"))
        ps_sm = ctx.enter_context(tc.tile_pool(name="ps_sm", bufs=3, space="PSUM"))

        mask_sb = consts.tile([P, P], b16)
        nc.sync.dma_start(mask_sb[:], mask)
        # Only what the first Q^T matmuls need is loaded up front; wkv/wo
        # are deferred below so they don't steal HBM bandwidth at the head.
        wq_sb = consts.tile([P, CCH, DL], b16)
        for dd in range(HPC):
            nc.scalar.dma_start(wq_sb[:, :, dd * P:(dd + 1) * P],
                                wq[:, :, dd * P:(dd + 1) * P])
        bq_sb = consts.tile([P, HPC], f32)
        nc.scalar.dma_start(bq_sb[:], bqv)
        id_sb = consts.tile([P, P], b16)
        nc.sync.dma_start(id_sb[:], ident)
        wkv_sb = consts.tile([P, CCH, 2 * DL], b16)
        wo_sb = consts.tile([P, HPC, D], b16)

        # Persistent per-(head) state, bf16: Q^T, K^T as [d, t]; V (+ones col)
        # as [t-blocks, d+1]; attn^T as [d, t].
        qT = persist.tile([P, HPC, T], b16)
        kT = persist.tile([P, HPC, T], b16)
        vA = persist.tile([P, HPC, T // P, HD + 4], b16)
        aT = persist.tile([P, HPC, T], b16)
        nc.vector.memset(vA[:, :, :, HD:HD + 1], 1.0)

        # ~4us of dummy matmuls to flip the PE HAM clock-gate to 8/8 while
        # the weight/x DMAs stream in.
        for w in range(40):
            psw = ps_sm.tile([P, HD + 4], f32, tag="sm")
            nc.tensor.matmul(psw[:, :P], mask_sb[:], mask_sb[:],
                             start=True, stop=True)

        for tj in range(NTCH):
            ts = slice(tj * TCH, (tj + 1) * TCH)
            # ---- x^T tiles for this t-chunk
            xts = []
            for cc in range(CCH):
                xt = xtp.tile([P, TCH], b16, tag="xt")
                nc.sync.dma_start(
                    xt[:], xT[cc * P:(cc + 1) * P, ts])
                xts.append(xt)

            # ---- Q^T projection: [d-block, t-chunk] += Wq_cc^T @ xT_cc
            for dd in range(HPC):
                psq = ps_mm.tile([P, TCH], f32, tag="mm")
                for cc in range(CCH):
                    nc.tensor.matmul(
                        psq[:], wq_sb[:, cc, dd * P:(dd + 1) * P], xts[cc][:],
                        start=(cc == 0), stop=(cc == CCH - 1))
                nc.vector.tensor_scalar(
                    qT[:, dd, ts], psq[:], bq_sb[:, dd:dd + 1], None, add)

            if tj == 0:
                nc.scalar.dma_start(wkv_sb[:], wkv)
                nc.scalar.dma_start(wo_sb[:], wo)

            # ---- K, V natural projection per t-block
            def flush_ktr(pending):
                for kk, ptb in pending:
                    for h in range(HPC):
                        pst = ps_sm.tile([P, 2 * (HD + 4)], b16, tag="sm")
                        nc.tensor.transpose(
                            pst[:, :P], kk[:, h * P:(h + 1) * P], id_sb[:])
                        nc.vector.tensor_copy(
                            out=kT[:, h, ptb * P:(ptb + 1) * P],
                            in_=pst[:, :P])
                pending.clear()

            pending_ktr = []
            for tl in range(TCH // P):
                tb = tj * (TCH // P) + tl
                bs = slice(tl * P, (tl + 1) * P)
                psk = ps_mm.tile([P, TCH], f32, tag="mm")
                psv = ps_mm.tile([P, TCH], f32, tag="mm")
                for cc in range(CCH):
                    nc.tensor.matmul(
                        psk[:], xts[cc][:, bs], wkv_sb[:, cc, :DL],
                        start=(cc == 0), stop=(cc == CCH - 1))
                for cc in range(CCH):
                    nc.tensor.matmul(
                        psv[:], xts[cc][:, bs], wkv_sb[:, cc, DL:],
                        start=(cc == 0), stop=(cc == CCH - 1))
                # k transposes of the PREVIOUS t-block: their k16 copy is
                # long done, so these matmuls issue without waiting on DVE.
                flush_ktr(pending_ktr)

                kf = kvf.tile([P, DL], f32, tag="kvf")
                nc.vector.tensor_copy(out=kf[:], in_=psk[:])
                nc.sync.dma_start(
                    k_out[:, tb * P:(tb + 1) * P, :].rearrange("h t d -> t h d"),
                    kf[:].rearrange("t (h d) -> t h d", d=HD))
                vf = kvf.tile([P, DL], f32, tag="kvf")
                nc.vector.tensor_copy(out=vf[:], in_=psv[:])
                nc.sync.dma_start(
                    v_out[:, tb * P:(tb + 1) * P, :].rearrange("h t d -> t h d"),
                    vf[:].rearrange("t (h d) -> t h d", d=HD))

                k16 = k16p.tile([P, DL], b16, tag="k16")
                nc.vector.tensor_copy(out=k16[:], in_=psk[:])
                nc.vector.tensor_copy(
                    out=vA[:, :, tb, :HD],
                    in_=psv[:].rearrange("t (h d) -> t h d", d=HD))
                pending_ktr.append((k16, tb))
            flush_ktr(pending_ktr)

            # ---- attention for q-chunk jq = tj (needs kv t-blocks <= this chunk)
            nkb = 4 * tj + 4

            def flush_atr(pending):
                for pan, ph, pg in pending:
                    pst = ps_sm.tile([P, 2 * (HD + 4)], b16, tag="sm")
                    nc.tensor.transpose(pst[:, :P], pan[:], id_sb[:])
                    nc.vector.tensor_copy(
                        out=aT[:, ph, pg * P:(pg + 1) * P], in_=pst[:, :P])
                pending.clear()

            pending_atr = []
            for h in range(HPC):
                ets = []
                for i in range(nkb):
                    pss = ps_sc.tile([P, TCH], f32, tag="sc")
                    nc.tensor.matmul(
                        pss[:], kT[:, h, i * P:(i + 1) * P], qT[:, h, ts],
                        start=True, stop=True)
                    et = expp.tile([P, TCH], b16, tag="exp")
                    nc.scalar.activation(et[:], pss[:], Exp)
                    if i >= 4 * tj:
                        dl = i - 4 * tj
                        nc.vector.tensor_tensor(
                            out=et[:, dl * P:(dl + 1) * P],
                            in0=et[:, dl * P:(dl + 1) * P],
                            in1=mask_sb[:], op=mult)
                    ets.append(et)
                # previous head's attn transposes: their normalize chain is
                # done by now, so these don't stall the PE stream.
                flush_atr(pending_atr)
                for gl in range(4):
                    g = 4 * tj + gl
                    psp = ps_sm.tile([P, HD + 4], f32, tag="sm")
                    for i in range(g + 1):
                        nc.tensor.matmul(
                            psp[:, :HD + 1],
                            ets[i][:, gl * P:(gl + 1) * P],
                            vA[:, h, i, :HD + 1],
                            start=(i == 0), stop=(i == g))
                    rec = recp.tile([P, 1], f32, tag="rec")
                    nc.vector.reciprocal(rec[:], psp[:, HD:HD + 1])
                    an = anp.tile([P, P], b16, tag="an")
                    nc.vector.tensor_scalar(
                        an[:], psp[:, :HD], rec[:], None, mult)
                    pending_atr.append((an, h, g))
            flush_atr(pending_atr)

            # ---- output projection, one chunk behind (fills attention
            # bubbles of the current chunk)
            wo_chunks = [tj - 1] if tj < NTCH - 1 else [tj - 1, tj]
            for wj in wo_chunks:
                if wj < 0:
                    continue
                for tl in range(TCH // P):
                    tb = wj * (TCH // P) + tl
                    for nn in range(D // TCH):
                        pso = ps_mm.tile([P, TCH], f32, tag="mm")
                        for h in range(HPC):
                            nc.tensor.matmul(
                                pso[:], aT[:, h, tb * P:(tb + 1) * P],
                                wo_sb[:, h, nn * TCH:(nn + 1) * TCH],
                                start=(h == 0), stop=(h == HPC - 1))
                        ob = outp.tile([P, TCH], f32, tag="ob")
                        nc.vector.tensor_copy(out=ob[:], in_=pso[:])
                        nc.sync.dma_start(
                            out_p[tb * P:(tb + 1) * P,
                                  nn * TCH:(nn + 1) * TCH],
                            ob[:])

    nc.compile()
    return nc


def _prep_inputs(x, Wq, bq, Wk, bk, Wv, bv, Wo, bo):
    scale = 1.0 / math.sqrt(HD)
    x = np.asarray(x, np.float32)
    xT_b = [np.ascontiguousarray(x[b].T).astype(BF16) for b in range(B)]

    ident = np.eye(P, dtype=np.float32).astype(BF16)
    # mask[k, q] = 1 where q >= k (upper triangular incl diagonal)
    mask = np.triu(np.ones((P, P), np.float32)).astype(BF16)

    per_group = []
    for hg in range(NHG):
        cols = slice(hg * DL, (hg + 1) * DL)
        wq_c = np.ascontiguousarray(
            (np.asarray(Wq[:, cols], np.float32) * scale)
            .reshape(CCH, P, DL).transpose(1, 0, 2)).astype(BF16)
        wkv_c = np.zeros((P, CCH, 2 * DL), np.float32)
        wkv_c[:, :, :DL] = np.asarray(Wk[:, cols], np.float32).reshape(
            CCH, P, DL).transpose(1, 0, 2)
        wkv_c[:, :, DL:] = np.asarray(Wv[:, cols], np.float32).reshape(
            CCH, P, DL).transpose(1, 0, 2)
        wkv_c = np.ascontiguousarray(wkv_c).astype(BF16)
        wo_c = np.ascontiguousarray(
            np.asarray(Wo[cols, :], np.float32)
            .reshape(HPC, HD, D).transpose(1, 0, 2)).astype(BF16)
        bq_c = np.ascontiguousarray(
            (np.asarray(bq[cols], np.float32) * scale).reshape(HPC, HD).T)
        per_group.append((wq_c, wkv_c, wo_c, bq_c))

    in_maps = []
    for c in range(NCORES):
        b, hg = c // NHG, c % NHG
        wq_c, wkv_c, wo_c, bq_c = per_group[hg]
        in_maps.append({
            "xT": xT_b[b], "wq": wq_c, "wkv": wkv_c, "wo": wo_c,
            "bqv": bq_c, "ident": ident, "mask": mask,
        })
    return in_maps


def _run(inputs, trace=False):
    import concourse.bass_utils as bass_utils

    if "nc" not in _CACHE:
        _CACHE["nc"] = _build_nc()
    nc = _CACHE["nc"]
    in_maps = _prep_inputs(**inputs)
    res = bass_utils.run_bass_kernel_spmd(
        nc, in_maps, core_ids=list(range(NCORES)), trace=trace)

    bo = np.asarray(inputs["bo"], np.float32)
    bk = np.asarray(inputs["bk"], np.float32)
    bv = np.asarray(inputs["bv"], np.float32)
    Wo = np.asarray(inputs["Wo"], np.float32)
    # K/V biases are skipped on device: softmax is shift-invariant in the
    # K bias, and sum(p)=1 makes the V bias contribute exactly bv @ Wo.
    out_bias = bo + bv @ Wo
    bk_h = bk.reshape(H, 1, HD)
    bv_h = bv.reshape(H, 1, HD)
    out = np.empty((B, T, D), np.float32)
    k = np.empty((B, H, T, HD), np.float32)
    v = np.empty((B, H, T, HD), np.float32)
    for b in range(B):
        acc = None
        for hg in range(NHG):
            r = res.results[b * NHG + hg]
            acc = r["out_p"] if acc is None else acc + r["out_p"]
            hs = slice(hg * HPC, (hg + 1) * HPC)
            k[b, hs] = r["k_out"] + bk_h[hs]
            v[b, hs] = r["v_out"] + bv_h[hs]
        out[b] = acc + out_bias
    return (out, k, v), res


def kernel(**inputs):
    outs, _ = _run(inputs, trace=False)
    return outs
